# revision 9
# baseline (speedup 1.0000x reference)
"""Trainium2 Bass kernel for nn_ColorGNN (bipartite GNN message passing).

Math restructuring
------------------
The reference builds a fully-connected bipartite edge list (E = B*C = 262144
edges) and runs a 2-layer MLP per edge. Because the graph is fully connected,
the edge-MLP first layer splits: ef @ W1 = (x_bird @ W1_top)[i] + (x_col @
W1_bot)[j], so the per-edge hidden is relu(A[i] + Bm[j] + b1) and the
scatter-add pushes through W2:
    aggr_bird[i]  = (sum_j relu(A[i]+Bm[j]+b1)) @ W2 + C*b2
    aggr_color[j] = (sum_i relu(A[i]+Bm[j]+b1)) @ W2 + B*b2
No E-sized tensor is ever materialized.

top_k with k=B=C selects every element, so filtered == probs and mask == 1;
the sort only permutes the final output.  The device computes combined =
(x_bird @ cpw + cpb) * probs; the host does the argsort/gather.

Sharding (8 cores)
------------------
Colors are sharded 64 per core. Each core keeps the full bird features
(replicated) and only its 64 colors' features. S_color is then complete
locally (sum over ALL birds); only S_bird needs one 256 KB AllReduce per
layer. All per-core differences enter through host-sliced input tensors, so
the program is pure SPMD.

Per-core layout: hidden dim H=128 lives on SBUF partitions; node features are
kept transposed (H x nodes). Inner loop per own-color t:
  T_t = relu(A + Bb[:, t])        (ScalarE activation w/ bias, or VectorE
                                   scalar_tensor_tensor, split between both)
  S_color[:, t] = sum_i T_t       (fused accum_out of the same instruction)
  S_bird += T_t                   (TensorE: identity matmul accumulating in
                                   PSUM)
"""

import numpy as np

import concourse.bass as bass
import concourse.bacc as bacc
import concourse.mybir as mybir
import concourse.tile as tile
from concourse import bass_utils

B = 512  # birds
C = 512  # colors
H = 128  # hidden
L = 3  # layers
NCORES = 8
CPC = C // NCORES  # colors per core

F32 = mybir.dt.float32
AF = mybir.ActivationFunctionType
ALU = mybir.AluOpType

# t-indices handled by ScalarE (rest go to VectorE). Contiguous halves so the
# two engines write disjoint S_color tiles.
# ScalarE tile: fused relu+reduce ~613ns. VectorE tile: tensor_scalar 2x
# (~330ns) + tensor_reduce 1x (~590ns) = ~920ns. Balance: 38/26.
N_ACT = 38


def _build(repeat: int = 1, variant: str = "full"):
    # variant: "v0" init+final only; "v1" layers w/ ACT-only loop, no accum;
    # "v2" ACT-only loop with accum_out; "full" ACT+DVE split with accum_out.
    nc = bacc.Bacc(
        "TRN2", target_bir_lowering=False, debug=False, num_devices=NCORES
    )

    # ---- I/O ----
    inp = {}

    def di(name, shape):
        inp[name] = nc.dram_tensor(name, list(shape), F32, kind="ExternalInput")
        return inp[name]

    di("probsT", (CPC, B))  # probs.T slice: my colors x all birds
    di("npw", (CPC, H))  # node_proj_w rows for my colors
    di("xc0", (H, CPC))  # initial color features (transposed)
    di("npb8", (H, 1))  # node_proj_b / 8
    di("ident", (H, H))
    for l in range(L):
        di(f"e1t{l}", (H, H))  # edge_w1[l][:H]
        di(f"e1b{l}", (H, H))  # edge_w1[l][H:]
        di(f"eb1_{l}", (H, 1))
        di(f"e2_{l}", (H, H))
        di(f"eb2f{l}", (H, 1))  # 512*edge_b2[l]
        di(f"eb2p{l}", (H, 1))  # 512*edge_b2[l] / 8
        di(f"n1x{l}", (H, H))  # node_w1[l][:H]
        di(f"n1a{l}", (H, H))  # node_w1[l][H:]
        di(f"nb1_{l}", (H, 1))
        di(f"n2_{l}", (H, H))
        di(f"nb2_{l}", (H, 1))
    di("cpw", (H, CPC))  # color_proj_w columns for my colors
    di("cpb", (CPC, 1))
    out_dram = nc.dram_tensor("out", [CPC, B], F32, kind="ExternalOutput")

    rg = [list(range(NCORES))]

    with tile.TileContext(nc) as tc:
        with (
            tc.tile_pool(name="const", bufs=1) as cpool,
            tc.tile_pool(name="work", bufs=2) as wpool,
            tc.tile_pool(name="tbuf", bufs=4) as tpool,
            tc.tile_pool(name="psA", bufs=2, space="PSUM") as psA,
            tc.tile_pool(name="psS", bufs=1, space="PSUM") as psS,
            tc.tile_pool(name="ps64", bufs=2, space="PSUM") as ps64,
            tc.tile_pool(name="dram", bufs=1, space="DRAM") as dpool,
        ):
            # ---- load constants to SBUF ----
            sb = {}
            for name, t in inp.items():
                s = cpool.tile(list(t.shape), F32, tag=name)
                nc.sync.dma_start(s[:], t[:])
                sb[name] = s

            zeros = cpool.tile([H, B], F32, tag="zeros")
            nc.vector.memset(zeros[:], 0.0)

            for _rep in range(repeat):
                # ---- init: x_bird^T = AllReduce(npw_mine^T @ probsT_mine) + npb
                ps = psA.tile([H, B], F32, tag="mm512")
                nc.tensor.matmul(ps[:], sb["npw"][:], sb["probsT"][:])
                xb_part = wpool.tile([H, B], F32, tag="xb_part")
                # + npb/8 on every core so the AllReduce sums to + npb
                nc.scalar.activation(
                    xb_part[:], ps[:], AF.Identity, bias=sb["npb8"][:]
                )
                cc_in = dpool.tile([H, B], F32, tag="cc_xb_in")
                cc_out = dpool.tile([H, B], F32, tag="cc_xb_out")
                nc.sync.dma_start(cc_in[:], xb_part[:])
                nc.gpsimd.collective_compute(
                    "AllReduce", ALU.add, replica_groups=rg,
                    ins=[cc_in[:]], outs=[cc_out[:]],
                )
                xbT = wpool.tile([H, B], F32, tag="xbT")
                nc.sync.dma_start(xbT[:], cc_out[:])
                xcT = sb["xc0"]

                for l in range(L if variant != "v0" else 0):
                    # A = relu-input contribution of birds: (H, B)
                    psa = psA.tile([H, B], F32, tag="mm512")
                    nc.tensor.matmul(psa[:], sb[f"e1t{l}"][:], xbT[:])
                    A_sb = wpool.tile([H, B], F32, tag="A_sb")
                    nc.scalar.copy(A_sb[:], psa[:])
                    # Bb = W1_bot^T @ xc_mine + eb1 : (H, CPC)
                    psb = ps64.tile([H, CPC], F32, tag="mm64")
                    nc.tensor.matmul(psb[:], sb[f"e1b{l}"][:], xcT[:])
                    Bb = wpool.tile([H, CPC], F32, tag="Bb")
                    nc.scalar.activation(
                        Bb[:], psb[:], AF.Identity, bias=sb[f"eb1_{l}"][:]
                    )

                    # ---- inner loop over my colors ----
                    ps_S = psS.tile([H, B], F32, tag="S")
                    scol_a = wpool.tile([H, N_ACT], F32, tag="scol_a")
                    scol_d = wpool.tile([H, CPC - N_ACT], F32, tag="scol_d")
                    if variant in ("v1", "v2", "v3"):
                        nc.vector.memset(scol_a[:], 0.0)
                        nc.vector.memset(scol_d[:], 0.0)
                    for t in range(CPC):
                        T_t = tpool.tile([H, B], F32, tag="T")
                        bias = Bb[:, t : t + 1]
                        if variant == "v1":
                            nc.scalar.activation(T_t[:], A_sb[:], AF.Relu, bias=bias)
                        elif variant == "v2":
                            col = (
                                scol_a[:, t : t + 1] if t < N_ACT
                                else scol_d[:, t - N_ACT : t - N_ACT + 1]
                            )
                            nc.scalar.activation(
                                T_t[:], A_sb[:], AF.Relu, bias=bias, accum_out=col
                            )
                        elif variant == "v3":
                            if t < N_ACT:
                                nc.scalar.activation(
                                    T_t[:], A_sb[:], AF.Relu, bias=bias,
                                    accum_out=scol_a[:, t : t + 1],
                                )
                            else:
                                nc.vector.scalar_tensor_tensor(
                                    out=T_t[:], in0=A_sb[:], scalar=bias,
                                    in1=zeros[:], op0=ALU.add, op1=ALU.max,
                                )
                        elif t < N_ACT:
                            nc.scalar.activation(
                                T_t[:], A_sb[:], AF.Relu, bias=bias,
                                accum_out=scol_a[:, t : t + 1],
                            )
                        else:
                            # stt-with-accum_out crashes HW; use tensor_scalar
                            # (2x fp32 mode) + explicit reduce instead.
                            nc.vector.tensor_scalar(
                                out=T_t[:], in0=A_sb[:], scalar1=bias,
                                scalar2=0.0, op0=ALU.add, op1=ALU.max,
                            )
                            nc.vector.tensor_reduce(
                                out=scol_d[:, t - N_ACT : t - N_ACT + 1],
                                in_=T_t[:], axis=mybir.AxisListType.X,
                                op=ALU.add,
                            )
                        nc.tensor.matmul(
                            ps_S[:], sb["ident"][:], T_t[:],
                            start=(t == 0), stop=(t == CPC - 1),
                        )

                    # ---- color side (fully local) ----
                    psac = ps64.tile([H, CPC], F32, tag="mm64")
                    nc.tensor.matmul(
                        psac[:, :N_ACT], sb[f"e2_{l}"][:], scol_a[:]
                    )
                    nc.tensor.matmul(
                        psac[:, N_ACT:], sb[f"e2_{l}"][:], scol_d[:]
                    )
                    aggrC = wpool.tile([H, CPC], F32, tag="aggrC")
                    nc.scalar.activation(
                        aggrC[:], psac[:], AF.Identity, bias=sb[f"eb2f{l}"][:]
                    )
                    # color node MLP
                    ph1c = ps64.tile([H, CPC], F32, tag="mm64")
                    nc.tensor.matmul(
                        ph1c[:], sb[f"n1x{l}"][:], xcT[:], start=True, stop=False
                    )
                    nc.tensor.matmul(
                        ph1c[:], sb[f"n1a{l}"][:], aggrC[:], start=False, stop=True
                    )
                    h1c = wpool.tile([H, CPC], F32, tag="h1c")
                    nc.scalar.activation(
                        h1c[:], ph1c[:], AF.Relu, bias=sb[f"nb1_{l}"][:]
                    )
                    px2c = ps64.tile([H, CPC], F32, tag="mm64")
                    nc.tensor.matmul(px2c[:], sb[f"n2_{l}"][:], h1c[:])
                    xcT_new = wpool.tile([H, CPC], F32, tag="xcT")
                    nc.scalar.activation(
                        xcT_new[:], px2c[:], AF.Identity, bias=sb[f"nb2_{l}"][:]
                    )

                    # ---- bird side: local W2 then AllReduce ----
                    S_sb = wpool.tile([H, B], F32, tag="S_sb")
                    nc.scalar.copy(S_sb[:], ps_S[:])
                    psab = psA.tile([H, B], F32, tag="mm512")
                    nc.tensor.matmul(psab[:], sb[f"e2_{l}"][:], S_sb[:])
                    aggrB_part = wpool.tile([H, B], F32, tag="aggrB_part")
                    nc.scalar.activation(
                        aggrB_part[:], psab[:], AF.Identity, bias=sb[f"eb2p{l}"][:]
                    )
                    ci = dpool.tile([H, B], F32, tag=f"cc{l}_in")
                    co = dpool.tile([H, B], F32, tag=f"cc{l}_out")
                    nc.sync.dma_start(ci[:], aggrB_part[:])
                    nc.gpsimd.collective_compute(
                        "AllReduce", ALU.add, replica_groups=rg,
                        ins=[ci[:]], outs=[co[:]],
                    )
                    aggrB = wpool.tile([H, B], F32, tag="aggrB")
                    nc.sync.dma_start(aggrB[:], co[:])
                    # bird node MLP (replicated)
                    ph1b = psA.tile([H, B], F32, tag="mm512")
                    nc.tensor.matmul(
                        ph1b[:], sb[f"n1x{l}"][:], xbT[:], start=True, stop=False
                    )
                    nc.tensor.matmul(
                        ph1b[:], sb[f"n1a{l}"][:], aggrB[:], start=False, stop=True
                    )
                    h1b = wpool.tile([H, B], F32, tag="h1b")
                    nc.scalar.activation(
                        h1b[:], ph1b[:], AF.Relu, bias=sb[f"nb1_{l}"][:]
                    )
                    px2b = psA.tile([H, B], F32, tag="mm512")
                    nc.tensor.matmul(px2b[:], sb[f"n2_{l}"][:], h1b[:])
                    xbT_new = wpool.tile([H, B], F32, tag="xbT")
                    nc.scalar.activation(
                        xbT_new[:], px2b[:], AF.Identity, bias=sb[f"nb2_{l}"][:]
                    )
                    xbT = xbT_new
                    xcT = xcT_new

                # ---- final: combined^T rows for my colors ----
                pssc = psA.tile([CPC, B], F32, tag="mm512")
                nc.tensor.matmul(pssc[:], sb["cpw"][:], xbT[:])
                out_sb = wpool.tile([CPC, B], F32, tag="out_sb")
                nc.vector.scalar_tensor_tensor(
                    out=out_sb[:], in0=pssc[:], scalar=sb["cpb"][:],
                    in1=sb["probsT"][:], op0=ALU.add, op1=ALU.mult,
                )
                nc.sync.dma_start(out_dram[:], out_sb[:])

    nc.compile()
    return nc


_BUILT = {}


def _get_built(repeat: int = 1):
    if repeat not in _BUILT:
        _BUILT[repeat] = _build(repeat)
    return _BUILT[repeat]


def make_in_maps(probs, node_proj_w, node_proj_b, edge_w1, edge_b1, edge_w2,
                 edge_b2, node_w1, node_b1, node_w2, node_b2, color_proj_w,
                 color_proj_b):
    f = lambda x: np.ascontiguousarray(np.asarray(x, dtype=np.float32))
    probs = f(probs)
    probsT = probs.T
    in_maps = []
    for c in range(NCORES):
        sl = slice(CPC * c, CPC * (c + 1))
        m = {
            "probsT": f(probsT[sl]),
            "npw": f(node_proj_w[sl]),
            "xc0": f((np.asarray(node_proj_w)[sl] + np.asarray(node_proj_b)).T),
            "npb8": f(np.asarray(node_proj_b) / 8.0).reshape(H, 1),
            "ident": np.eye(H, dtype=np.float32),
            "cpw": f(np.asarray(color_proj_w)[:, sl]),
            "cpb": f(np.asarray(color_proj_b)[sl]).reshape(CPC, 1),
        }
        for l in range(L):
            m[f"e1t{l}"] = f(edge_w1[l][:H])
            m[f"e1b{l}"] = f(edge_w1[l][H:])
            m[f"eb1_{l}"] = f(edge_b1[l]).reshape(H, 1)
            m[f"e2_{l}"] = f(edge_w2[l])
            m[f"eb2f{l}"] = f(512.0 * np.asarray(edge_b2[l])).reshape(H, 1)
            m[f"eb2p{l}"] = f(512.0 * np.asarray(edge_b2[l]) / 8.0).reshape(H, 1)
            m[f"n1x{l}"] = f(node_w1[l][:H])
            m[f"n1a{l}"] = f(node_w1[l][H:])
            m[f"nb1_{l}"] = f(node_b1[l]).reshape(H, 1)
            m[f"n2_{l}"] = f(node_w2[l])
            m[f"nb2_{l}"] = f(node_b2[l]).reshape(H, 1)
        in_maps.append(m)
    return in_maps


def finish(probs, core_outs):
    combinedT = np.concatenate(core_outs, axis=0)  # (C, B)
    combined = combinedT.T  # (B, C)
    probs = np.asarray(probs, dtype=np.float32)
    idx = np.argsort(-probs, axis=1, kind="stable")
    cost = 1.0 - np.take_along_axis(combined, idx, axis=1)
    return cost.astype(np.float32)


def kernel(probs, node_proj_w, node_proj_b, edge_w1, edge_b1, edge_w2,
           edge_b2, node_w1, node_b1, node_w2, node_b2, color_proj_w,
           color_proj_b):
    nc = _get_built()
    in_maps = make_in_maps(
        probs, node_proj_w, node_proj_b, edge_w1, edge_b1, edge_w2, edge_b2,
        node_w1, node_b1, node_w2, node_b2, color_proj_w, color_proj_b,
    )
    res = bass_utils.run_bass_kernel_spmd(nc, in_maps, list(range(NCORES)))
    return finish(probs, [r["out"] for r in res.results])
